# revision 6
# baseline (speedup 1.0000x reference)
"""Fused self-modulated-LN transformer block (MHA + MLP) on 8 trn2 NeuronCores.

Sharding: core c handles (batch b = c//2, query-half = c%2).  Each core:
  - computes sln1 on its 512 query rows, projects q for all 16 heads,
  - projects k/v for its full batch element (1024 rows, all heads),
  - computes scores/softmax/attnV for all heads on its 512 query rows,
  - writes its slice of the attn-probs output directly,
  - fc + residual + sln2 + MLP on its 512 rows, writes its out slice.
No cross-core communication.  Matmuls run as float32r (full-rate fp32).
"""

import numpy as np

import concourse.bass as bass
import concourse.mybir as mb
from concourse import tile
from concourse.bass_utils import run_bass_kernel_spmd

B, L, D = 4, 1024, 1024
H, DK, DV = 16, 64, 64
HID = 4 * D
EPS = 1e-5
NCORES = 8
TOK = 512  # query tokens per core
SCALE = 1.0 / float(np.sqrt(DK))

F32 = mb.dt.float32
F32R = mb.dt.float32r
AF = mb.ActivationFunctionType
ALU = mb.AluOpType


def _split_excess_waits(nc, max_waits=1):
    """walrus (CoreV3) rejects instructions carrying multiple sync waits;
    move extras onto NOPs inserted just before the offending instruction."""
    n = 0
    for f in nc.m.functions:
        for bb in f.blocks:
            il = bb.instructions
            i = 0
            while i < len(il):
                inst = il[i]
                si = inst.sync_info
                if si and si.on_wait and len(si.on_wait) > max_waits:
                    waits = list(si.on_wait)
                    extra, keep = waits[:-max_waits], waits[-max_waits:]
                    for w in extra:
                        nop = mb.InstNoOp(name=f"I-ws{nc.next_id()}", ins=[], outs=[])
                        nop.engine = inst.engine
                        nop.sync_info = mb.SyncInfo(on_wait=[w], on_update=[])
                        il.insert(i, nop)
                        i += 1
                        n += 1
                    inst.sync_info = mb.SyncInfo(
                        on_wait=keep, on_update=list(si.on_update)
                    )
                i += 1
    return n


def _build(uniform1: bool, uniform2: bool):
    nc = bass.Bass()

    # ---- I/O ----
    q_in = nc.dram_tensor("q_in", [TOK, D], F32, kind="ExternalInput")
    z_in = nc.dram_tensor("z_in", [TOK, D], F32, kind="ExternalInput")
    kT_in = nc.dram_tensor("kT_in", [D, L], F32R, kind="ExternalInput")
    vT_in = nc.dram_tensor("vT_in", [D, L], F32R, kind="ExternalInput")
    wq = nc.dram_tensor("wq", [D, H * DK], F32R, kind="ExternalInput")
    wk = nc.dram_tensor("wk", [D, H * DK], F32R, kind="ExternalInput")
    wv = nc.dram_tensor("wv", [D, H * DV], F32R, kind="ExternalInput")
    fcw = nc.dram_tensor("fcw", [H * DV, D], F32R, kind="ExternalInput")
    m1w = nc.dram_tensor("m1w", [D, HID], F32R, kind="ExternalInput")
    m2w = nc.dram_tensor("m2w", [HID, D], F32R, kind="ExternalInput")
    wqb = nc.dram_tensor("wqb", [H * DK], F32, kind="ExternalInput")
    wkb = nc.dram_tensor("wkb", [H * DK], F32, kind="ExternalInput")
    wvb = nc.dram_tensor("wvb", [H * DV], F32, kind="ExternalInput")
    fcb = nc.dram_tensor("fcb", [D], F32, kind="ExternalInput")
    m1b = nc.dram_tensor("m1b", [HID], F32, kind="ExternalInput")
    m2b = nc.dram_tensor("m2b", [D], F32, kind="ExternalInput")
    a1 = nc.dram_tensor("a1", [1 if uniform1 else D], F32, kind="ExternalInput")
    c1 = nc.dram_tensor("c1", [1 if uniform1 else D], F32, kind="ExternalInput")
    a2 = nc.dram_tensor("a2", [1 if uniform2 else D], F32, kind="ExternalInput")
    c2 = nc.dram_tensor("c2", [1 if uniform2 else D], F32, kind="ExternalInput")
    ident = nc.dram_tensor("ident", [128, 128], F32, kind="ExternalInput")

    attn_sh = nc.dram_tensor("attn_sh", [H, TOK, L], F32, kind="ExternalOutput")
    out_sh = nc.dram_tensor("out_sh", [TOK, D], F32, kind="ExternalOutput")

    def bcast(ap):
        # [n] DRAM AP broadcast across 128 partitions -> [128, n]
        return bass.AP(
            tensor=ap.tensor, offset=ap.offset, ap=[[0, 128]] + list(ap.ap)
        )

    with tile.TileContext(nc) as tc:
        with tc.tile_pool(name="const", bufs=1) as const, \
             tc.tile_pool(name="pz", bufs=1) as pz, \
             tc.tile_pool(name="pq1T", bufs=1) as pq1T, \
             tc.tile_pool(name="poutT", bufs=1) as poutT:

            # ---- constants ----
            ident_sb = const.tile([128, 128], F32)
            nc.gpsimd.dma_start(out=ident_sb, in_=ident[:, :])
            eps_sb = const.tile([128, 1], F32)
            nc.vector.memset(eps_sb, EPS)
            wqb_sb = const.tile([128, 8], F32)
            nc.gpsimd.dma_start(out=wqb_sb, in_=wqb.rearrange("(c p) -> p c", p=128))
            wkb_sb = const.tile([128, 8], F32)
            nc.gpsimd.dma_start(out=wkb_sb, in_=wkb.rearrange("(c p) -> p c", p=128))
            fcb_sb = const.tile([128, 8], F32)
            nc.gpsimd.dma_start(out=fcb_sb, in_=fcb.rearrange("(c p) -> p c", p=128))
            m1b_sb = const.tile([128, 32], F32)
            nc.gpsimd.dma_start(out=m1b_sb, in_=m1b.rearrange("(c p) -> p c", p=128))
            m2b_sb = const.tile([128, 8], F32)
            nc.gpsimd.dma_start(out=m2b_sb, in_=m2b.rearrange("(c p) -> p c", p=128))
            wvb_bc = const.tile([128, H * DV], F32)
            nc.gpsimd.dma_start(out=wvb_bc, in_=bcast(wvb[:]))
            a1_sb = const.tile([128, 1 if uniform1 else D], F32)
            nc.gpsimd.dma_start(out=a1_sb, in_=bcast(a1[:]))
            c1_sb = const.tile([128, 1 if uniform1 else D], F32)
            nc.gpsimd.dma_start(out=c1_sb, in_=bcast(c1[:]))
            a2_sb = const.tile([128, 1 if uniform2 else D], F32)
            nc.gpsimd.dma_start(out=a2_sb, in_=bcast(a2[:]))
            c2_sb = const.tile([128, 1 if uniform2 else D], F32)
            nc.gpsimd.dma_start(out=c2_sb, in_=bcast(c2[:]))

            # ---- long-lived activations ----
            z_tm = pz.tile([128, 4, D], F32)  # z, token-major
            nc.sync.dma_start(out=z_tm, in_=z_in.rearrange("(t p) d -> p t d", p=128))
            q1T = pq1T.tile([128, 8, TOK], F32R)  # sln1(q), feature-major
            outT = poutT.tile([128, 8, TOK], F32)  # post-fc residual, feat-major

            def sln(x_tm, out_tm_t, a_sb, c_sb, uniform, pool):
                """out = z * (A * LN0(x) + C), token-major [128, 4, D]."""
                for t in range(4):
                    xs = x_tm[:, t, :]
                    stats = pool.tile([128, 2, 6], F32, tag="ln_stats")
                    for j in range(2):
                        nc.vector.bn_stats(
                            out=stats[:, j, :], in_=xs[:, j * 512:(j + 1) * 512])
                    mv = pool.tile([128, 2], F32, tag="ln_mv")
                    nc.vector.bn_aggr(out=mv, in_=stats)
                    sd = pool.tile([128, 1], F32, tag="ln_sd")
                    nc.scalar.activation(
                        out=sd, in_=mv[:, 1:2], func=AF.Sqrt, bias=eps_sb, scale=1.0)
                    rstd = pool.tile([128, 1], F32, tag="ln_rstd")
                    nc.vector.reciprocal(out=rstd, in_=sd)
                    if uniform:
                        # s = rstd*A ; c = -(mean*s) + C ; y = (x*s + c) * z
                        s_ = pool.tile([128, 1], F32, tag="ln_s")
                        nc.vector.tensor_mul(s_, rstd, a_sb)
                        cc = pool.tile([128, 1], F32, tag="ln_c")
                        nc.vector.tensor_mul(cc, mv[:, 0:1], s_)
                        nc.vector.scalar_tensor_tensor(
                            out=cc, in0=cc, scalar=-1.0, in1=c_sb,
                            op0=ALU.mult, op1=ALU.add)
                        t1 = pool.tile([128, D], F32, tag="ln_t1")
                        nc.vector.tensor_scalar(
                            out=t1, in0=xs, scalar1=s_, scalar2=cc,
                            op0=ALU.mult, op1=ALU.add)
                        nc.vector.tensor_mul(out_tm_t[:, t, :], t1, z_tm[:, t, :])
                    else:
                        negm = pool.tile([128, 1], F32, tag="ln_negm")
                        nc.vector.tensor_mul(negm, mv[:, 0:1], rstd)
                        nc.vector.tensor_scalar_mul(negm, negm, -1.0)
                        n_ = pool.tile([128, D], F32, tag="ln_n")
                        nc.vector.tensor_scalar(
                            out=n_, in0=xs, scalar1=rstd, scalar2=negm,
                            op0=ALU.mult, op1=ALU.add)
                        t1 = pool.tile([128, D], F32, tag="ln_t1")
                        nc.vector.tensor_mul(t1, n_, a_sb)
                        nc.vector.tensor_add(t1, t1, c_sb)
                        nc.vector.tensor_mul(out_tm_t[:, t, :], t1, z_tm[:, t, :])

            def transpose_to_fmajor(src_tm, dst_T, pspool, n_t=4, n_dc=8):
                # [128, n_t, n_dc*128] token-major -> [128, n_dc, n_t*128]
                for dc in range(n_dc):
                    pst = pspool.tile([128, n_t * 128], F32, tag="tp")
                    for t in range(n_t):
                        nc.tensor.transpose(
                            pst[:, t * 128:(t + 1) * 128],
                            src_tm[:, t, dc * 128:(dc + 1) * 128],
                            ident_sb,
                        )
                    nc.vector.tensor_copy(dst_T[:, dc, :], pst)

            # ---- phase A: sln1(q) + transpose ----
            with tc.tile_pool(name="phA", bufs=2) as pool, \
                 tc.tile_pool(name="phA_ps", bufs=2, space="PSUM") as pspool:
                q_tm = pool.tile([128, 4, D], F32, tag="q_tm", bufs=1)
                nc.sync.dma_start(
                    out=q_tm, in_=q_in.rearrange("(t p) d -> p t d", p=128))
                q1_tm = pool.tile([128, 4, D], F32, tag="q1_tm", bufs=1)
                sln(q_tm, q1_tm, a1_sb, c1_sb, uniform1, pool)
                transpose_to_fmajor(q1_tm, q1T, pspool)

            with tc.tile_pool(name="pkvq", bufs=1) as pkvq:
                khT = pkvq.tile([128, 8, L], F32R)      # k-proj, feature-major
                vh = pkvq.tile([128, 8, H * DV], F32R)  # v-proj, token-major
                qhT = pkvq.tile([128, 8, TOK], F32R)    # q-proj, feature-major

                # ---- phase B1: k projection (feature-major) ----
                with tc.tile_pool(name="phBk", bufs=1) as pool, \
                     tc.tile_pool(name="phBk_ps", bufs=4, space="PSUM") as pspool:
                    wk_sb = pool.tile([128, 8, H * DK], F32R)
                    nc.sync.dma_start(
                        out=wk_sb, in_=wk.rearrange("(c p) n -> p c n", p=128))
                    kT_sb = pool.tile([128, 8, L], F32R)
                    nc.sync.dma_start(
                        out=kT_sb, in_=kT_in.rearrange("(c p) n -> p c n", p=128))
                    for oc in range(8):
                        for th in range(2):
                            ps = pspool.tile([128, 512], F32, tag="kps")
                            for dc in range(8):
                                nc.tensor.matmul(
                                    ps,
                                    wk_sb[:, dc, oc * 128:(oc + 1) * 128],
                                    kT_sb[:, dc, th * 512:(th + 1) * 512],
                                    start=(dc == 0), stop=(dc == 7),
                                )
                            nc.scalar.activation(
                                out=khT[:, oc, th * 512:(th + 1) * 512], in_=ps,
                                func=AF.Identity, bias=wkb_sb[:, oc:oc + 1],
                                scale=1.0)

                # ---- phase B2: v projection (token-major) ----
                with tc.tile_pool(name="phBv", bufs=1) as pool, \
                     tc.tile_pool(name="phBv_ps", bufs=4, space="PSUM") as pspool:
                    wv_sb = pool.tile([128, 8, H * DV], F32R)
                    nc.sync.dma_start(
                        out=wv_sb, in_=wv.rearrange("(c p) n -> p c n", p=128))
                    vT_sb = pool.tile([128, 8, L], F32R)
                    nc.sync.dma_start(
                        out=vT_sb, in_=vT_in.rearrange("(c p) n -> p c n", p=128))
                    for oc in range(8):  # kv token chunks
                        for dh in range(2):
                            ps = pspool.tile([128, 512], F32, tag="vps")
                            for dc in range(8):
                                nc.tensor.matmul(
                                    ps,
                                    vT_sb[:, dc, oc * 128:(oc + 1) * 128],
                                    wv_sb[:, dc, dh * 512:(dh + 1) * 512],
                                    start=(dc == 0), stop=(dc == 7),
                                )
                            nc.vector.tensor_add(
                                vh[:, oc, dh * 512:(dh + 1) * 512], ps,
                                wvb_bc[:, dh * 512:(dh + 1) * 512])

                # ---- phase B3: q projection (feature-major) ----
                with tc.tile_pool(name="phBq", bufs=1) as pool, \
                     tc.tile_pool(name="phBq_ps", bufs=4, space="PSUM") as pspool:
                    wq_sb = pool.tile([128, 8, H * DK], F32R)
                    nc.sync.dma_start(
                        out=wq_sb, in_=wq.rearrange("(c p) n -> p c n", p=128))
                    for oc in range(8):
                        ps = pspool.tile([128, TOK], F32, tag="qps")
                        for dc in range(8):
                            nc.tensor.matmul(
                                ps,
                                wq_sb[:, dc, oc * 128:(oc + 1) * 128],
                                q1T[:, dc, :],
                                start=(dc == 0), stop=(dc == 7),
                            )
                        nc.scalar.activation(
                            out=qhT[:, oc, :], in_=ps,
                            func=AF.Identity, bias=wqb_sb[:, oc:oc + 1], scale=1.0)

                # ---- phase C: attention ----
                with tc.tile_pool(name="pao", bufs=1) as pao:
                    attnout_tm = pao.tile([128, 4, H * DV], F32)
                    with tc.tile_pool(name="phC", bufs=2) as pool, \
                         tc.tile_pool(name="phC_T", bufs=2) as poolT, \
                         tc.tile_pool(name="phC_psA", bufs=2, space="PSUM") as psA_pool, \
                         tc.tile_pool(name="phC_psT", bufs=2, space="PSUM") as psT_pool, \
                         tc.tile_pool(name="phC_psV", bufs=2, space="PSUM") as psV_pool:
                        for h in range(H):
                            po = (h % 2) * 64
                            hc = h // 2
                            lq = qhT[po:po + 64, hc, :]  # [64, 512] d-major
                            lk = khT[po:po + 64, hc, :]  # [64, 1024] d-major
                            recips = []
                            for qc in range(4):
                                psA = psA_pool.tile([128, L], F32, tag="psA")
                                for kc in range(2):
                                    nc.tensor.matmul(
                                        psA[:, kc * 512:(kc + 1) * 512],
                                        lq[:, qc * 128:(qc + 1) * 128],
                                        lk[:, kc * 512:(kc + 1) * 512],
                                        start=True, stop=True,
                                    )
                                probs_u = pool.tile([128, L], F32, tag="probs_u")
                                sums = pool.tile([128, 1], F32, tag="sums", bufs=8)
                                nc.scalar.activation(
                                    out=probs_u, in_=psA, func=AF.Exp, scale=SCALE,
                                    accum_out=sums)
                                recip = pool.tile([128, 1], F32, tag="recip", bufs=8)
                                nc.vector.reciprocal(out=recip, in_=sums)
                                recips.append(recip)
                                probs_n = pool.tile([128, L], F32, tag="probs_n")
                                nc.vector.tensor_scalar_mul(probs_n, probs_u, recip)
                                nc.sync.dma_start(
                                    out=attn_sh[h, qc * 128:(qc + 1) * 128, :],
                                    in_=probs_n)
                            # transposed (unnormalized) probs for attnV
                            probsT = poolT.tile([128, 8, TOK], F32R, tag="probsT")
                            for kc in range(8):
                                psT = psT_pool.tile([128, TOK], F32, tag="psT")
                                nc.tensor.matmul(
                                    psT,
                                    lk[:, kc * 128:(kc + 1) * 128],
                                    lq,
                                    start=True, stop=True,
                                )
                                nc.scalar.activation(
                                    out=probsT[:, kc, :], in_=psT, func=AF.Exp,
                                    scale=SCALE)
                            for qc in range(4):
                                psV = psV_pool.tile([128, DV], F32, tag="psV")
                                for kc in range(8):
                                    nc.tensor.matmul(
                                        psV,
                                        probsT[:, kc, qc * 128:(qc + 1) * 128],
                                        vh[:, kc, h * DV:(h + 1) * DV],
                                        start=(kc == 0), stop=(kc == 7),
                                    )
                                nc.vector.tensor_scalar_mul(
                                    attnout_tm[:, qc, h * DV:(h + 1) * DV],
                                    psV, recips[qc])

                    # ---- phase D: fc + residual (feature-major) ----
                    with tc.tile_pool(name="phD", bufs=1) as pool, \
                         tc.tile_pool(name="phD_ps", bufs=4, space="PSUM") as pspool:
                        attnoutT = pool.tile([128, 8, TOK], F32R)
                        transpose_to_fmajor(attnout_tm, attnoutT, pspool)
                        fc_sb = pool.tile([128, 8, D], F32R)
                        nc.sync.dma_start(
                            out=fc_sb, in_=fcw.rearrange("(c p) n -> p c n", p=128))
                        for oc in range(8):
                            ps = pspool.tile([128, TOK], F32, tag="fcps")
                            for dc in range(8):
                                nc.tensor.matmul(
                                    ps,
                                    fc_sb[:, dc, oc * 128:(oc + 1) * 128],
                                    attnoutT[:, dc, :],
                                    start=(dc == 0), stop=(dc == 7),
                                )
                            # outT = ps + fc_b + q1T (residual)
                            nc.vector.scalar_tensor_tensor(
                                out=outT[:, oc, :], in0=ps,
                                scalar=fcb_sb[:, oc:oc + 1],
                                in1=q1T[:, oc, :].bitcast(F32), op0=ALU.add, op1=ALU.add)

            # ---- phase E: sln2 + MLP ----
            with tc.tile_pool(name="ph2T", bufs=1) as p2T, \
                 tc.tile_pool(name="phMg", bufs=1) as gpool:
                h2T = p2T.tile([128, 8, TOK], F32R)
                gh = gpool.tile([128, 32, TOK], F32R)  # gelu(mlp1), feature-major

                with tc.tile_pool(name="phE", bufs=1) as pool, \
                     tc.tile_pool(name="phE_ln", bufs=2) as lnpool, \
                     tc.tile_pool(name="phE_ps", bufs=2, space="PSUM") as pspool:
                    out_tm = pool.tile([128, 4, D], F32)
                    for t in range(4):
                        pst = pspool.tile([128, D], F32, tag="tp2")
                        for dc in range(8):
                            nc.tensor.transpose(
                                pst[:, dc * 128:(dc + 1) * 128],
                                outT[:, dc, t * 128:(t + 1) * 128],
                                ident_sb,
                            )
                        nc.vector.tensor_copy(out_tm[:, t, :], pst)
                    h2_tm = pool.tile([128, 4, D], F32)
                    sln(out_tm, h2_tm, a2_sb, c2_sb, uniform2, lnpool)
                    transpose_to_fmajor(h2_tm, h2T, pspool)

                with tc.tile_pool(name="phM1", bufs=2) as m1pool, \
                     tc.tile_pool(name="phM1_ps", bufs=2, space="PSUM") as ps1pool:
                    for hb in range(8):
                        m1_sb = m1pool.tile([128, 8, 512], F32R, tag="m1w")
                        nc.sync.dma_start(
                            out=m1_sb,
                            in_=m1w[:, hb * 512:(hb + 1) * 512].rearrange(
                                "(c p) n -> p c n", p=128))
                        for j in range(4):
                            hcg = hb * 4 + j
                            ps = ps1pool.tile([128, TOK], F32, tag="m1ps")
                            for dc in range(8):
                                nc.tensor.matmul(
                                    ps,
                                    m1_sb[:, dc, j * 128:(j + 1) * 128],
                                    h2T[:, dc, :],
                                    start=(dc == 0), stop=(dc == 7),
                                )
                            nc.scalar.activation(
                                out=gh[:, hcg, :], in_=ps, func=AF.Gelu,
                                bias=m1b_sb[:, hcg:hcg + 1], scale=1.0)

                with tc.tile_pool(name="phF", bufs=1) as fpool:
                    finT = fpool.tile([128, 8, TOK], F32)
                    with tc.tile_pool(name="phM2", bufs=4) as m2pool, \
                         tc.tile_pool(name="phM2_ps", bufs=1, space="PSUM") as ps2pool:
                        ps_out = [
                            ps2pool.tile([128, TOK], F32, tag=f"m2ps{oc}", bufs=1, name=f"m2ps{oc}")
                            for oc in range(8)
                        ]
                        for hcg in range(32):
                            m2_sb = m2pool.tile([128, D], F32R, tag="m2w")
                            nc.sync.dma_start(
                                out=m2_sb, in_=m2w[hcg * 128:(hcg + 1) * 128, :])
                            for oc in range(8):
                                nc.tensor.matmul(
                                    ps_out[oc],
                                    m2_sb[:, oc * 128:(oc + 1) * 128],
                                    gh[:, hcg, :],
                                    start=(hcg == 0), stop=(hcg == 31),
                                )
                        for oc in range(8):
                            nc.vector.scalar_tensor_tensor(
                                out=finT[:, oc, :], in0=ps_out[oc],
                                scalar=m2b_sb[:, oc:oc + 1], in1=outT[:, oc, :],
                                op0=ALU.add, op1=ALU.add)
                    with tc.tile_pool(name="phF_ps", bufs=2, space="PSUM") as psf:
                        fin_tm = fpool.tile([128, 4, D], F32)
                        for t in range(4):
                            pst = psf.tile([128, D], F32, tag="tp3")
                            for dc in range(8):
                                nc.tensor.transpose(
                                    pst[:, dc * 128:(dc + 1) * 128],
                                    finT[:, dc, t * 128:(t + 1) * 128],
                                    ident_sb,
                                )
                            nc.vector.tensor_copy(fin_tm[:, t, :], pst)
                        nc.sync.dma_start(
                            out=out_sh.rearrange("(t p) d -> p t d", p=128),
                            in_=fin_tm)

    _split_excess_waits(nc)
    return nc


_CACHE = {}


def _get_nc(uniform1, uniform2):
    key = (uniform1, uniform2)
    if key not in _CACHE:
        _CACHE[key] = _build(uniform1, uniform2)
    return _CACHE[key]


def _uniform(x):
    return bool(np.all(x == x.flat[0]))


def _prep_inputs(q, k, v, z,
                 wq_w, wq_b, wk_w, wk_b, wv_w, wv_b, fc_w, fc_b,
                 mlp1_w, mlp1_b, mlp2_w, mlp2_b,
                 ln1_w, ln1_b, g1, b1, ln2_w, ln2_b, g2, b2):
    f = lambda x: np.ascontiguousarray(np.asarray(x, dtype=np.float32))
    q, k, v, z = f(q), f(k), f(v), f(z)
    g1s, b1s = float(np.asarray(g1).ravel()[0]), float(np.asarray(b1).ravel()[0])
    g2s, b2s = float(np.asarray(g2).ravel()[0]), float(np.asarray(b2).ravel()[0])
    A1 = f(g1s * np.asarray(ln1_w))
    C1 = f(g1s * np.asarray(ln1_b) + b1s)
    A2 = f(g2s * np.asarray(ln2_w))
    C2 = f(g2s * np.asarray(ln2_b) + b2s)
    u1 = _uniform(A1) and _uniform(C1)
    u2 = _uniform(A2) and _uniform(C2)
    shared = {
        "wq": f(wq_w), "wk": f(wk_w), "wv": f(wv_w), "fcw": f(fc_w),
        "m1w": f(mlp1_w), "m2w": f(mlp2_w),
        "wqb": f(wq_b), "wkb": f(wk_b), "wvb": f(wv_b), "fcb": f(fc_b),
        "m1b": f(mlp1_b), "m2b": f(mlp2_b),
        "a1": A1[:1].copy() if u1 else A1,
        "c1": C1[:1].copy() if u1 else C1,
        "a2": A2[:1].copy() if u2 else A2,
        "c2": C2[:1].copy() if u2 else C2,
        "ident": np.eye(128, dtype=np.float32),
    }
    kT = [np.ascontiguousarray(k[b].T) for b in range(B)]
    vT = [np.ascontiguousarray(v[b].T) for b in range(B)]
    in_maps = []
    for c in range(NCORES):
        b, half = c // 2, c % 2
        m = dict(shared)
        m["q_in"] = np.ascontiguousarray(q[b, half * TOK:(half + 1) * TOK, :])
        m["z_in"] = np.ascontiguousarray(z[b, half * TOK:(half + 1) * TOK, :])
        m["kT_in"] = kT[b]
        m["vT_in"] = vT[b]
        in_maps.append(m)
    return in_maps, u1, u2


def _assemble(results):
    out = np.empty((B, L, D), np.float32)
    attn = np.empty((H * B, L, L), np.float32)
    for c in range(NCORES):
        b, half = c // 2, c % 2
        r = results[c]
        out[b, half * TOK:(half + 1) * TOK, :] = r["out_sh"]
        attn[b::B, half * TOK:(half + 1) * TOK, :] = r["attn_sh"]
    return out, attn


def kernel(**inputs):
    in_maps, u1, u2 = _prep_inputs(**inputs)
    nc = _get_nc(u1, u2)
    res = run_bass_kernel_spmd(nc, in_maps, list(range(NCORES)))
    return _assemble(res.results)
